# revision 1
# baseline (speedup 1.0000x reference)
"""Multi-head attention forward on 8 TRN2 NeuronCores.

Problem: B=2, L=2048, D=1024, H=16, Hd=64 MHA block:
    qkv = x @ w_qkv + b_qkv ; per-head softmax(q k^T / sqrt(Hd)) @ v ; o @ w_out + b_out

Sharding (tensor parallel over heads x batch):
  core c -> batch c//4, heads [4*(c%4), 4*(c%4)+4).
  Each core computes its 4 heads' attention for its batch and a partial
  out-projection, emitted transposed in bf16 (y^T, (1024, 2048)). Host
  transposes and sums the 4 partials per batch + b_out in fp32.

v2 design notes (driven by the v1 trace):
  * ScalarE's exp stream is the wall: 128 x [128,1024] EXP instrs ~ 147us.
    The whole schedule is built to keep that stream dense: attention is
    processed in 512-query slices; each key-block's two head-pair scores
    land in one [128,1024] PSUM tile (2 banks) so one EXP covers both.
  * Score matmuls for the head pair use K=64 row groups 0-63 / 64-127 and
    are emitted back-to-back so they run concurrently in the PE array.
  * All projection / out-projection work is interleaved into the attention
    emission as PE filler so the PE never idles long enough for the HAM
    clock monitor to re-throttle it to 1.2 GHz (v1 lost ~45us to that).
  * Out-projection is weight-stationary (y^T = wo^T @ o^T), streamed per
    query-slice so the y DMA overlaps attention instead of tailing.
  * Normalization divides straight out of PSUM (reciprocal of the
    accumulated denominator row + broadcast via DRAM bounce + multiply),
    no intermediate copies.
  * PSUM budget: scores [128,1024]x2 (4 banks) + po [65,1024] (2) +
    proj/outproj [128,512]x2 (2) = 8 banks exactly.
"""

from contextlib import ExitStack

import numpy as np

B, L, D = 2, 2048, 1024
H, HD = 16, 64
NCORES = 8
CORES_PER_BATCH = 4
H_C = H // CORES_PER_BATCH          # heads per core = 4
COLS = H_C * HD                     # qkv cols per core = 256
P = 128
NKT = D // P                        # 8 contraction tiles over D
NQS = L // 512                      # 4 query slices of 512
NKB = L // P                        # 16 key/token blocks of 128
NMB = COLS // P                     # 2 col-blocks of the per-core qkv slice
NDT = COLS // P                     # 2 contraction tiles over per-core o dims
NDB = D // P                        # 8 out-dim blocks of y^T
SCALE = 1.0 / np.sqrt(np.float32(HD))

_NC_CACHE = None
LAST_RESULTS = None


def _build_nc():
    import os
    import concourse.bass as bass
    import concourse.tile as tile
    from concourse import bacc, mybir

    dbg = bool(os.environ.get("KDEBUG"))

    f32 = mybir.dt.float32
    bf16 = mybir.dt.bfloat16
    Exp = mybir.ActivationFunctionType.Exp

    nc = bacc.Bacc(None, target_bir_lowering=False)

    xt_d = nc.declare_dram_parameter("xt", [NKT, P, L], bf16, isOutput=False)
    wq_d = nc.declare_dram_parameter("wq", [NKT, P, COLS], bf16, isOutput=False)
    wk_d = nc.declare_dram_parameter("wk", [NKT, P, COLS], bf16, isOutput=False)
    wv_d = nc.declare_dram_parameter("wv", [NKT, P, COLS], bf16, isOutput=False)
    bq_d = nc.declare_dram_parameter("bq", [NMB, P, 1], f32, isOutput=False)
    bk_d = nc.declare_dram_parameter("bk", [NMB, P, 1], f32, isOutput=False)
    bv_d = nc.declare_dram_parameter("bv", [1, COLS], bf16, isOutput=False)
    wo_d = nc.declare_dram_parameter("wo", [NDT, P, D], bf16, isOutput=False)
    yt_d = nc.declare_dram_parameter("yt", [D, L], bf16, isOutput=True)
    if dbg:
        qt_dump = nc.declare_dram_parameter("qt_dump", [P, NMB, L], bf16, isOutput=True)
        kt_dump = nc.declare_dram_parameter("kt_dump", [P, NMB, L], bf16, isOutput=True)
        vx_dump = nc.declare_dram_parameter("vx_dump", [P, NKB, H_C, HD + 1], bf16, isOutput=True)
        ot_dump = nc.declare_dram_parameter("ot_dump", [P, NDT, L], bf16, isOutput=True)
        p_dump = nc.declare_dram_parameter("p_dump", [P, 1024], bf16, isOutput=True)
        o65_dump = nc.declare_dram_parameter("o65_dump", [HD + 1, 1024], f32, isOutput=True)
        rec_dump = nc.declare_dram_parameter("rec_dump", [1, 1024], f32, isOutput=True)
        pb_dump = nc.declare_dram_parameter("pb_dump", [HD, 1024], f32, isOutput=True)

    with tile.TileContext(nc) as tc, ExitStack() as ctx, nc.allow_low_precision(
        "bf16 matmul operands; accumulation stays fp32 in PSUM"
    ):
        consts = ctx.enter_context(tc.tile_pool(name="consts", bufs=1))
        xtp = ctx.enter_context(tc.tile_pool(name="xtp", bufs=NKT))
        wp = ctx.enter_context(tc.tile_pool(name="wp", bufs=NKT))
        bigs = ctx.enter_context(tc.tile_pool(name="bigs", bufs=1))
        pp = ctx.enter_context(tc.tile_pool(name="pp", bufs=3))
        yp = ctx.enter_context(tc.tile_pool(name="yp", bufs=3))
        smallp = ctx.enter_context(tc.tile_pool(name="smallp", bufs=2))
        drp = ctx.enter_context(tc.tile_pool(name="drp", bufs=2, space="DRAM"))
        # PSUM (8 banks): scores [128,1024]f32 x2 bufs = 4 banks,
        # po [65,1024] x1 = 2 banks, proj/outproj [128,512] x2 = 2 banks.
        psum_s = ctx.enter_context(tc.tile_pool(name="psum_s", bufs=2, space="PSUM"))
        psum_po = ctx.enter_context(tc.tile_pool(name="psum_po", bufs=1, space="PSUM"))
        psum_y = ctx.enter_context(tc.tile_pool(name="psum_y", bufs=2, space="PSUM"))

        # ---- constants / ACT table warm ----
        warm_f = consts.tile([1, 8], f32, tag="warm_f")
        nc.vector.memset(warm_f[:], 0.0)
        warm_o = consts.tile([1, 8], bf16, tag="warm_o")
        nc.scalar.activation(warm_o[:], warm_f[:], Exp)

        ones_f32 = consts.tile([1, P], f32, tag="ones_f32")
        nc.vector.memset(ones_f32[:], 1.0)
        ones_sb = consts.tile([1, P], bf16, tag="ones")
        nc.vector.tensor_copy(ones_sb[:], ones_f32[:])
        bq_sb = consts.tile([P, NMB], f32, tag="bq")
        bk_sb = consts.tile([P, NMB], f32, tag="bk")
        for mb in range(NMB):
            nc.sync.dma_start(out=bq_sb[:, mb : mb + 1], in_=bq_d[mb])
            nc.sync.dma_start(out=bk_sb[:, mb : mb + 1], in_=bk_d[mb])
        bv_sb = consts.tile([1, COLS], bf16, tag="bv")
        nc.sync.dma_start(out=bv_sb[:], in_=bv_d[:])

        # ---- input DMAs: wk/wq first, then x quarters (v weights early) ----
        xt_t = [xtp.tile([P, L], bf16, tag="xt", name=f"xt{i}") for i in range(NKT)]
        wq_t = [wp.tile([P, COLS], bf16, tag="wq", name=f"wq{i}") for i in range(NKT)]
        wk_t = [wp.tile([P, COLS], bf16, tag="wk", name=f"wk{i}") for i in range(NKT)]
        wv_t = [wp.tile([P, COLS], bf16, tag="wv", name=f"wv{i}") for i in range(NKT)]
        wo_t = [wp.tile([P, D], bf16, tag="wo", name=f"wo{i}", bufs=NDT) for i in range(NDT)]
        qs_engines = [nc.sync, nc.scalar, nc.gpsimd]
        qi = 0

        def dma_in(out, in_):
            nonlocal qi
            qs_engines[qi % len(qs_engines)].dma_start(out=out, in_=in_)
            qi += 1

        # output/bounce DMA triggers stay off the scalar queue (exp stream)
        yq_engines = [nc.sync, nc.gpsimd]
        yqi = 0

        def dma_out(out, in_):
            nonlocal yqi
            yq_engines[yqi % len(yq_engines)].dma_start(out=out, in_=in_)
            yqi += 1

        # phase-ordered: everything the first attention window needs, first
        for kt in range(NKT):
            dma_in(wk_t[kt][:], wk_d[kt])
        for kt in range(NKT):
            dma_in(xt_t[kt][:, 0:512], xt_d[kt][:, 0:512])
        for kt in range(NKT):
            dma_in(wq_t[kt][:], wq_d[kt])
        for kt in range(NKT):
            dma_in(wv_t[kt][:], wv_d[kt])
        for quarter in range(1, 4):
            sl = slice(quarter * (L // 4), (quarter + 1) * (L // 4))
            for kt in range(NKT):
                dma_in(xt_t[kt][:, sl], xt_d[kt][:, sl])
            if quarter == 1:
                for dt_i in range(NDT):
                    dma_in(wo_t[dt_i][:], wo_d[dt_i])

        # ---- persistent intermediates ----
        qt_sb = bigs.tile([P, NMB, L], bf16, tag="qt")
        kt_sb = bigs.tile([P, NMB, L], bf16, tag="kt")
        vx_sb = bigs.tile([P, NKB, H_C, HD + 1], bf16, tag="vx")
        vxones_f32 = consts.tile([P, NKB, H_C, 1], f32, tag="vxones")
        nc.vector.memset(vxones_f32[:], 1.0)
        nc.vector.tensor_copy(vx_sb[:, :, :, HD : HD + 1], vxones_f32[:])
        ot_sb = bigs.tile([P, NDT, L], bf16, tag="ot")

        # ---- filler emitters (PE work fed into the attention stream) ----
        # Emitted in ~1us chunks so a filler pop between attention
        # kb-iterations never delays the next score matmuls by much.
        def proj_qk_chunks(w_t, b_sb, dst, mb, sl4):
            """project 512 tokens [sl4*512, ...) of q^T or k^T col-block mb."""
            ps = psum_y.tile([P, 512], f32, tag="y", name="ps_qk")
            tok = slice(sl4 * 512, (sl4 + 1) * 512)

            def chunk(k0, k1):
                for kt in range(k0, k1):
                    nc.tensor.matmul(
                        ps,
                        lhsT=w_t[kt][:, mb * P : (mb + 1) * P],
                        rhs=xt_t[kt][:, tok],
                        start=(kt == 0),
                        stop=(kt == NKT - 1),
                    )
                if k1 == NKT:
                    nc.vector.tensor_scalar_add(
                        dst[:, mb, tok], ps, b_sb[:, mb : mb + 1]
                    )

            return [lambda: chunk(0, 3), lambda: chunk(3, 6), lambda: chunk(6, NKT)]

        def proj_v_chunks(tb):
            """project token block tb of v (all 4 heads), augmented layout."""
            ps = psum_y.tile([P, 512], f32, tag="y", name="ps_v")[:, :COLS]

            def chunk(k0, k1):
                for kt in range(k0, k1):
                    nc.tensor.matmul(
                        ps,
                        lhsT=xt_t[kt][:, tb * P : (tb + 1) * P],
                        rhs=wv_t[kt][:],
                        start=(kt == 0),
                        stop=False,
                    )
                if k1 == NKT:
                    nc.tensor.matmul(
                        ps, lhsT=ones_sb[:], rhs=bv_sb[:], start=False, stop=True
                    )
                    nc.vector.tensor_copy(
                        vx_sb[:, tb, :, 0:HD],
                        ps.rearrange("p (h d) -> p h d", h=H_C),
                    )

            return [lambda: chunk(0, 4), lambda: chunk(4, NKT)]

        def out_proj(qs, dblk, tail=False):
            """y^T[dblk*128:, qs*512:] = wo[:, dblk]^T @ o^T   (weight-stationary)"""
            tok = slice(qs * 512, (qs + 1) * 512)
            ps = psum_y.tile([P, 512], f32, tag="y", name="ps_yt")
            for dt_i in range(NDT):
                nc.tensor.matmul(
                    ps,
                    lhsT=wo_t[dt_i][:, dblk * P : (dblk + 1) * P],
                    rhs=ot_sb[:, dt_i, tok],
                    start=(dt_i == 0),
                    stop=(dt_i == NDT - 1),
                )
            y_sb = yp.tile([P, 512], bf16, tag="y_sb", name="y_sb")
            # after the exp stream ends, ScalarE is idle: run the tail
            # copies there so they overlap the DVE's final normalize
            if tail:
                nc.scalar.copy(y_sb[:], ps)
            else:
                nc.vector.tensor_copy(y_sb[:], ps)
            dma_out(yt_d[dblk * P : (dblk + 1) * P, tok], y_sb[:])

        # Filler queue: keyed chunk-closures (~1us of PE work each), popped
        # between attention kb-iterations.  Ordered so data deps (DMA
        # arrival, attention prerequisites) are met just in time; force()
        # emits all chunks of a specific prerequisite immediately.
        filler = []

        def add_filler(key, chunks):
            filler.extend((key, c) for c in chunks)

        def pop_filler(n):
            for _ in range(n):
                if filler:
                    filler.pop(0)[1]()

        def force(key):
            i = 0
            while i < len(filler):
                if filler[i][0] == key:
                    filler.pop(i)[1]()
                else:
                    i += 1

        def add_k(mb, s):
            add_filler(("k", mb, s), proj_qk_chunks(wk_t, bk_sb, kt_sb, mb, s))

        def add_q(mb, s):
            add_filler(("q", mb, s), proj_qk_chunks(wq_t, bq_sb, qt_sb, mb, s))

        # chase order: quarter0-gated work first, then later quarters
        add_k(0, 0)
        add_q(0, 0)
        for tb in range(2):
            add_filler(("v", tb), proj_v_chunks(tb))
        for s in range(1, 4):
            add_k(0, s)
        for tb in range(2, NKB):
            add_filler(("v", tb), proj_v_chunks(tb))
        for s in range(4):
            add_k(1, s)
        add_q(1, 0)

        # ---- prefix: minimum to start the exp stream ----
        # (ordered by DMA arrival: wk+xq0 gate k, +wq gates q, +wv gates v.
        # v(0)/v(1) MUST be prefix-forced: deferring them puts a wv-DMA-gated
        # matmul inside the in-order PE stream and stalls everything behind)
        force(("k", 0, 0))
        force(("q", 0, 0))
        force(("v", 0))
        force(("v", 1))

        # ---- main attention stream ----
        def attention(qs, mb):
            # force prerequisites that are still sitting in the filler queue
            force(("q", mb, qs))
            tokq = slice(qs * 512, (qs + 1) * 512)
            po = psum_po.tile([HD + 1, 1024], f32, tag="po", name="po")
            pend = []  # (kb, p_tile) awaiting PV

            def emit_pv(kb, p_t):
                force(("v", kb))
                for hh in range(2):
                    nc.tensor.matmul(
                        po[:, hh * 512 : (hh + 1) * 512],
                        lhsT=vx_sb[:, kb, 2 * mb + hh, :],
                        rhs=p_t[:, hh * 512 : (hh + 1) * 512],
                        start=(kb == 0),
                        stop=(kb == NKB - 1),
                    )

            for kb in range(NKB):
                force(("k", mb, kb // 4))
                ps = psum_s.tile([P, 1024], f32, tag="s", name="ps_s")
                for hh in range(2):
                    off = hh * HD
                    nc.tensor.matmul(
                        ps[:, hh * 512 : (hh + 1) * 512],
                        lhsT=kt_sb[off : off + HD, mb, kb * P : (kb + 1) * P],
                        rhs=qt_sb[off : off + HD, mb, tokq],
                        start=True,
                        stop=True,
                    )
                p_t = pp.tile([P, 1024], bf16, tag="p", name="p_t")
                nc.scalar.activation(p_t[:], ps[:], Exp)
                if dbg and qs == 0 and mb == 0 and kb == 0:
                    nc.sync.dma_start(out=p_dump[:], in_=p_t[:])
                pend.append((kb, p_t))
                if len(pend) > 1:
                    emit_pv(*pend.pop(0))
                pop_filler(1)
            for kb_p, p_p in pend:
                emit_pv(kb_p, p_p)

            # normalize: o65 <- po (o rows + denominator row); broadcast the
            # denominator row over 64 partitions with a K=1 ones-matmul
            # (no DRAM bounce), reciprocal partition-aligned (a partition-
            # shifted custom-DVE reciprocal silently corrupts on HW), then
            # o * rec -> ot.
            # normalize: o65 <- po (o rows + denominator row); bounce the
            # denominator row through DRAM to broadcast it over 64
            # partitions (no PE involvement -> no PE pipeline bubble),
            # reciprocal partition-aligned (a partition-shifted custom-DVE
            # reciprocal silently corrupts on HW), then o * rec -> ot.
            o65 = smallp.tile([HD + 1, 1024], f32, tag="o65", name="o65")
            nc.vector.tensor_copy(o65[:], po[:])
            sums_dr = drp.tile([1, 1024], f32, tag="sums_dr", name="sums_dr")
            nc.sync.dma_start(out=sums_dr[:], in_=o65[HD : HD + 1, :])
            sums_bc = smallp.tile([HD, 1024], f32, tag="sums_bc", name="sums_bc")
            dr_ap = sums_dr[:]
            nc.sync.dma_start(
                out=sums_bc[:],
                in_=bass.AP(
                    tensor=dr_ap.tensor,
                    offset=dr_ap.offset,
                    ap=[[0, HD], dr_ap.ap[-1]],
                ),
            )
            pb = smallp.tile([HD, 1024], f32, tag="pb", name="pb")
            nc.vector.reciprocal_approx_fast(pb[:], sums_bc[:])
            if dbg and qs == 0 and mb == 0:
                nc.sync.dma_start(out=o65_dump[:], in_=o65[:])
                nc.sync.dma_start(out=pb_dump[:], in_=pb[:])
            for hh in range(2):
                off = hh * HD
                nc.vector.tensor_mul(
                    ot_sb[off : off + HD, mb, tokq],
                    o65[0:HD, hh * 512 : (hh + 1) * 512],
                    pb[:, hh * 512 : (hh + 1) * 512],
                )

        tail_reserve = []
        for qs in range(NQS):
            for mb in range(NMB):
                attention(qs, mb)
                # stage next-round prerequisites as filler.  qs0's windows
                # are already oversubscribed by the v/k chase, so only
                # qs1's own q goes there; qs2/qs3's q-projections wait for
                # qs1's windows (which have spare PE capacity).
                if qs == 0 and mb == 0:
                    add_q(0, 1)
                    add_q(1, 1)
                if qs == 1 and mb == 0:
                    for mq, sq in [(0, 2), (1, 2), (0, 3), (1, 3)]:
                        add_q(mq, sq)
            # out-projection of this slice becomes filler for the next slice;
            # half of qs2's is held back so the PE has work during the final
            # normalize (else HAM re-throttles and the last slice runs cold)
            if qs < NQS - 1:
                for dd in range(NDB):
                    if qs == 2 and dd >= 4:
                        tail_reserve.append(lambda q=qs, d=dd: out_proj(q, d, tail=True))
                    else:
                        add_filler(("yo", qs, dd), [lambda q=qs, d=dd: out_proj(q, d)])
        # drain: remaining filler, reserved PE work, last slice
        while filler:
            filler.pop(0)[1]()
        for c in tail_reserve:
            c()
        for d in range(NDB):
            out_proj(NQS - 1, d, tail=True)
        if dbg:
            nc.sync.dma_start(out=qt_dump[:], in_=qt_sb[:])
            nc.sync.dma_start(out=kt_dump[:], in_=kt_sb[:])
            nc.sync.dma_start(out=vx_dump[:], in_=vx_sb[:])
            nc.sync.dma_start(out=ot_dump[:], in_=ot_sb[:])

    nc.finalize()
    return nc


def get_nc():
    global _NC_CACHE
    if _NC_CACHE is None:
        _NC_CACHE = _build_nc()
    return _NC_CACHE


def make_in_maps(x, w_qkv, b_qkv, w_out):
    import ml_dtypes

    bf16 = ml_dtypes.bfloat16
    x = np.asarray(x, dtype=np.float32)
    w_qkv = np.asarray(w_qkv, dtype=np.float32)
    b_qkv = np.asarray(b_qkv, dtype=np.float32)
    w_out = np.asarray(w_out, dtype=np.float32)

    in_maps = []
    for c in range(NCORES):
        b, g = divmod(c, CORES_PER_BATCH)
        cs, ce = g * COLS, (g + 1) * COLS
        xt = np.ascontiguousarray(x[b].T).reshape(NKT, P, L).astype(bf16)
        wq = (w_qkv[:, 0 * D : 1 * D][:, cs:ce] * SCALE).reshape(NKT, P, COLS).astype(bf16)
        wk = np.ascontiguousarray(w_qkv[:, 1 * D : 2 * D][:, cs:ce]).reshape(NKT, P, COLS).astype(bf16)
        wv = np.ascontiguousarray(w_qkv[:, 2 * D : 3 * D][:, cs:ce]).reshape(NKT, P, COLS).astype(bf16)
        bq = np.ascontiguousarray(b_qkv[0 * D : 1 * D][cs:ce] * SCALE).reshape(
            NMB, P, 1
        )
        bk = np.ascontiguousarray(b_qkv[1 * D : 2 * D][cs:ce]).reshape(NMB, P, 1)
        bv = b_qkv[2 * D : 3 * D][cs:ce].reshape(1, COLS).astype(bf16)
        wo = np.ascontiguousarray(w_out[cs:ce, :]).reshape(NDT, P, D).astype(bf16)
        in_maps.append(
            dict(xt=xt, wq=wq, wk=wk, wv=wv, bq=bq, bk=bk, bv=bv, wo=wo)
        )
    return in_maps


def kernel(x, w_qkv, b_qkv, w_out, b_out, _trace=False, **_kw):
    global LAST_RESULTS
    import os
    # Reset cores on runtime init: clears accumulated device state from
    # prior activity (observed to inflate exec time 240us -> 286us until
    # reset). No effect on a clean device; respect an explicit setting.
    os.environ.setdefault("NEURON_RT_RESET_CORES", "1")
    from concourse.bass_utils import run_bass_kernel_spmd

    nc = get_nc()
    in_maps = make_in_maps(x, w_qkv, b_qkv, w_out)
    res = run_bass_kernel_spmd(nc, in_maps, list(range(NCORES)), trace=_trace, **_kw)
    LAST_RESULTS = res

    b_out = np.asarray(b_out, dtype=np.float32)
    y = np.zeros((B, L, D), dtype=np.float32)
    for c in range(NCORES):
        y[c // CORES_PER_BATCH] += res.results[c]["yt"].astype(np.float32).T
    y += b_out[None, None, :]
    return y

